# revision 1
# baseline (speedup 1.0000x reference)
"""Self-contained GAT-2-layer kernel for Trainium2 (8 NeuronCores, SPMD).

kernel(**inputs) takes the FULL unsharded inputs of nn_GAT_13400297964010
(x [100000,128] f32, edge_index [2,1600000] int, weights/biases) and
returns the full [100000, 32] f32 output, computed on 8 TRN2 cores.
"""
"""GAT 2-layer Bass kernel for TRN2, 8-core SPMD.

Sharding: nodes split into 8 contiguous dst-ranges (one per core); edges
(incl. self-loops) sorted by dst; each core owns all edges into its range,
grouped into 128-node dst windows, padded to TMAX 128-edge tiles/window.

Per core:
  Phase A: h_ext = x @ W1ext for ALL nodes -> DRAM [Npad, 136]
           (cols 0:128 h in (c,h)-interleave, 128:132 as1, 132:136 ad1;
           ad1 also copied to compact table).
  Phase B (L1): per window, accumulate psum[m, 136] over tiles:
           G = h_ext[src] (indirect gather), ind_em = (iota == dst_rel),
           ind_me = transpose(ind_em), ad_e = ind_me.T @ ad_win,
           w = exp(lrelu(as+ad)), F = G*w, psum += ind_em.T @ [F|junk|w].
  Flush:   h1 = numer/denom + b1, relu; h2 = h1T.T @ W2ext -> h2 shard
           (+ ad2 shard).
  AllGather shards -> full h2_ext / ad2 tables.
  Phase C (L2): same pipeline on 36-col records, 1 head -> out rows.
"""
import numpy as np
from contextlib import ExitStack

import concourse.bass as bass
import concourse.bacc as bacc
import concourse.mybir as mybir
from concourse.tile import TileContext

F32 = mybir.dt.float32
I32 = mybir.dt.int32
AF = mybir.ActivationFunctionType
OP = mybir.AluOpType

HEADS, HID, OUT_CH, IN_CH = 4, 32, 32, 128
REC1 = 136
REC2 = 36
NEG = 0.2


class Cfg:
    def __init__(self, N, n_cores=8, TMAX=19, WIN=128):
        self.N = N
        self.n_cores = n_cores
        self.npc = N // n_cores
        assert self.npc * n_cores == N
        self.WIN = WIN
        self.nwin = (self.npc + WIN - 1) // WIN
        self.win_sizes = [min(WIN, self.npc - w * WIN) for w in range(self.nwin)]
        self.TMAX = TMAX
        self.Npad = ((N + 127) // 128) * 128
        self.ntiles_a = self.Npad // 128
        self.S = self.nwin * self.TMAX * 128


def build_gat(cfg: Cfg):
    nc = bacc.Bacc("TRN2", target_bir_lowering=False, debug=False,
                   num_devices=cfg.n_cores)

    xT = nc.declare_dram_parameter("xT", [IN_CH, cfg.Npad], F32, isOutput=False)
    w1ext = nc.declare_dram_parameter("w1ext", [IN_CH, REC1], F32, isOutput=False)
    w2ext = nc.declare_dram_parameter("w2ext", [128, 34], F32, isOutput=False)
    b1r = nc.declare_dram_parameter("b1r", [128, 128], F32, isOutput=False)
    b2r = nc.declare_dram_parameter("b2r", [128, 32], F32, isOutput=False)
    iotar = nc.declare_dram_parameter("iotar", [128, 128], F32, isOutput=False)
    identr = nc.declare_dram_parameter("identr", [128, 128], F32, isOutput=False)
    onesr = nc.declare_dram_parameter("onesr", [128, 1], F32, isOutput=False)
    msrc = nc.declare_dram_parameter("msrc", [cfg.S, 1], I32, isOutput=False)
    mrel = nc.declare_dram_parameter("mrel", [cfg.S, 1], F32, isOutput=False)
    wrows = nc.declare_dram_parameter("wrows", [cfg.nwin * 128, 1], I32, isOutput=False)
    out = nc.declare_dram_parameter("out", [cfg.npc, 32], F32, isOutput=True)

    h_ext = nc.dram_tensor("h_ext", [cfg.Npad, REC1], F32)
    ad1t = nc.dram_tensor("ad1t", [cfg.Npad, 4], F32)
    h2sh = nc.dram_tensor("h2sh", [cfg.npc, REC2], F32)
    ad2sh = nc.dram_tensor("ad2sh", [cfg.npc, 4], F32)
    h2full = nc.dram_tensor("h2full", [cfg.N, REC2], F32, addr_space="Shared")
    ad2full = nc.dram_tensor("ad2full", [cfg.N, 4], F32, addr_space="Shared")

    rg = [list(range(cfg.n_cores))]

    with TileContext(nc) as tc, ExitStack() as top:
        cpool = top.enter_context(tc.tile_pool(name="consts", bufs=1))

        def cload(shape, src, tag):
            t = cpool.tile(shape, F32, tag=tag)
            nc.sync.dma_start(out=t[:], in_=src[:])
            return t

        w1_sb = cload([IN_CH, REC1], w1ext, "w1c")
        w2_sb = cload([128, 34], w2ext, "w2c")
        b1_sb = cload([128, 128], b1r, "b1c")
        b2_sb = cload([128, 32], b2r, "b2c")
        iota_sb = cload([128, 128], iotar, "iotac")
        id_sb = cload([128, 128], identr, "identc")
        ones_sb = cload([128, 1], onesr, "onesc")
        wrows_sb = cpool.tile([128, cfg.nwin], I32)
        nc.sync.dma_start(out=wrows_sb[:],
                          in_=wrows[:, 0].rearrange("(w p) -> p w", p=128))

        # ---------------- Phase A ----------------
        with tc.tile_pool(name="pa_sb", bufs=4) as pa_sb, \
             tc.tile_pool(name="pa_ps", bufs=4, space="PSUM") as pa_ps:
            for t in range(cfg.ntiles_a):
                xt = pa_sb.tile([IN_CH, 128], F32, tag="xt")
                nc.sync.dma_start(out=xt[:], in_=xT[:, t * 128:(t + 1) * 128])
                hp = pa_ps.tile([128, REC1], F32, tag="hp")
                nc.tensor.matmul(out=hp[:], lhsT=xt[:], rhs=w1_sb[:],
                                 start=True, stop=True)
                hs = pa_sb.tile([128, REC1], F32, tag="hs")
                nc.vector.tensor_copy(out=hs[:], in_=hp[:])
                nc.sync.dma_start(out=h_ext[t * 128:(t + 1) * 128, :], in_=hs[:])
                nc.sync.dma_start(out=ad1t[t * 128:(t + 1) * 128, :],
                                  in_=hs[:, 132:136])

        # ---------------- edge phases ----------------
        def edge_phase(layer, table, ad_tab, rec, nheads):
            wrec = rec if layer == 1 else 34
            with tc.tile_pool(name=f"e{layer}m", bufs=3) as mpool, \
                 tc.tile_pool(name=f"e{layer}s", bufs=8) as spool, \
                 tc.tile_pool(name=f"e{layer}i", bufs=8) as ipool, \
                 tc.tile_pool(name=f"e{layer}f", bufs=2) as fpool, \
                 tc.tile_pool(name=f"e{layer}w", bufs=2, space="PSUM") as wps, \
                 tc.tile_pool(name=f"e{layer}t", bufs=2, space="PSUM") as tps, \
                 tc.tile_pool(name=f"e{layer}a", bufs=2, space="PSUM") as aps:
                for w in range(cfg.nwin):
                    wsz = cfg.win_sizes[w]
                    base = w * cfg.TMAX * 128
                    srcw = mpool.tile([128, cfg.TMAX], I32, tag="srcw")
                    nc.sync.dma_start(
                        out=srcw[:],
                        in_=msrc[base:base + cfg.TMAX * 128, 0]
                        .rearrange("(t p) -> p t", p=128))
                    relw = mpool.tile([128, cfg.TMAX], F32, tag="relw")
                    nc.sync.dma_start(
                        out=relw[:],
                        in_=mrel[base:base + cfg.TMAX * 128, 0]
                        .rearrange("(t p) -> p t", p=128))
                    adw = mpool.tile([128, 4], F32, tag="adw")
                    nc.gpsimd.indirect_dma_start(
                        out=adw[:, :nheads] if nheads == 4 else adw[:, 0:nheads],
                        out_offset=None, in_=ad_tab[:],
                        in_offset=bass.IndirectOffsetOnAxis(
                            ap=wrows_sb[:, w:w + 1], axis=0))

                    winp = wps.tile([128, wrec], F32, tag="winp")
                    for t in range(cfg.TMAX):
                        g = spool.tile([128, rec], F32, tag="g")
                        nc.gpsimd.indirect_dma_start(
                            out=g[:], out_offset=None, in_=table[:],
                            in_offset=bass.IndirectOffsetOnAxis(
                                ap=srcw[:, t:t + 1], axis=0))
                        ind_em = ipool.tile([128, 128], F32, tag="ind_em")
                        nc.vector.tensor_scalar(
                            out=ind_em[:], in0=iota_sb[:],
                            scalar1=relw[:, t:t + 1], scalar2=None,
                            op0=OP.is_equal)
                        ime_ps = tps.tile([128, 128], F32, tag="ime")
                        nc.tensor.transpose(out=ime_ps[:], in_=ind_em[:],
                                            identity=id_sb[:])
                        ind_me = ipool.tile([128, 128], F32, tag="ind_me")
                        nc.scalar.activation(out=ind_me[:], in_=ime_ps[:],
                                             func=AF.Copy)
                        adp = aps.tile([128, 4], F32, tag="adp")
                        nc.tensor.matmul(out=adp[:, :nheads], lhsT=ind_me[:],
                                         rhs=adw[:, :nheads], start=True, stop=True)
                        if layer == 1:
                            asl, wsl = g[:, 128:132], g[:, 132:136]
                        else:
                            asl, wsl = g[:, 32:33], g[:, 34:35]
                        esum = spool.tile([128, 4], F32, tag="esum")
                        nc.vector.tensor_tensor(out=esum[:, :nheads], in0=asl,
                                                in1=adp[:, :nheads], op=OP.add)
                        elr = spool.tile([128, 4], F32, tag="elr")
                        nc.vector.tensor_scalar(out=elr[:, :nheads],
                                                in0=esum[:, :nheads],
                                                scalar1=NEG, scalar2=None,
                                                op0=OP.mult)
                        nc.vector.tensor_tensor(out=elr[:, :nheads],
                                                in0=esum[:, :nheads],
                                                in1=elr[:, :nheads],
                                                op=OP.max)
                        nc.scalar.activation(out=wsl, in_=elr[:, :nheads],
                                             func=AF.Exp)
                        if layer == 1:
                            wap = bass.AP(wsl.tensor, wsl.offset,
                                          [wsl.ap[0], [0, 32], [1, 4]])
                            nc.vector.tensor_tensor(out=g[:, 0:128],
                                                    in0=g[:, 0:128], in1=wap,
                                                    op=OP.mult)
                        else:
                            nc.vector.tensor_scalar(out=g[:, 0:34],
                                                    in0=g[:, 0:34], scalar1=wsl,
                                                    scalar2=None, op0=OP.mult)
                        nc.tensor.matmul(out=winp[:], lhsT=ind_em[:],
                                         rhs=g[:, 0:wrec],
                                         start=(t == 0),
                                         stop=(t == cfg.TMAX - 1))

                    r0 = w * 128
                    if layer == 1:
                        denr = fpool.tile([128, 4], F32, tag="denr")
                        nc.vector.reciprocal(out=denr[:], in_=winp[:, 132:136])
                        h1 = fpool.tile([128, 128], F32, tag="h1")
                        pv = winp[:, 0:128].rearrange("p (c h) -> p h c", h=4)
                        hv = h1[:].rearrange("p (c h) -> p h c", h=4)
                        for hh in range(4):
                            nc.vector.tensor_scalar(
                                out=hv[:, hh, :], in0=pv[:, hh, :],
                                scalar1=denr[:, hh:hh + 1], scalar2=None,
                                op0=OP.mult)
                        nc.vector.tensor_tensor(out=h1[:], in0=h1[:],
                                                in1=b1_sb[:], op=OP.add)
                        nc.scalar.activation(out=h1[:], in_=h1[:], func=AF.Relu)
                        h1tp = tps.tile([128, 128], F32, tag="ime")
                        nc.tensor.transpose(out=h1tp[:], in_=h1[:],
                                            identity=id_sb[:])
                        h1t = fpool.tile([128, 128], F32, tag="h1t")
                        nc.scalar.activation(out=h1t[:], in_=h1tp[:], func=AF.Copy)
                        h2p = aps.tile([128, 34], F32, tag="adp")
                        nc.tensor.matmul(out=h2p[:], lhsT=h1t[:], rhs=w2_sb[:],
                                         start=True, stop=True)
                        h2s = fpool.tile([128, 34], F32, tag="h2s")
                        nc.vector.tensor_copy(out=h2s[:], in_=h2p[:])
                        nc.sync.dma_start(out=h2sh[r0:r0 + wsz, 0:33],
                                          in_=h2s[:wsz, 0:33])
                        nc.sync.dma_start(out=h2sh[r0:r0 + wsz, 33:34],
                                          in_=ones_sb[:wsz, :])
                        nc.sync.dma_start(out=ad2sh[r0:r0 + wsz, 0:1],
                                          in_=h2s[:wsz, 33:34])
                    else:
                        denr = fpool.tile([128, 1], F32, tag="denr2")
                        nc.vector.reciprocal(out=denr[:], in_=winp[:, 33:34])
                        o = fpool.tile([128, 32], F32, tag="o")
                        nc.vector.tensor_scalar(out=o[:], in0=winp[:, 0:32],
                                                scalar1=denr[:], scalar2=None,
                                                op0=OP.mult)
                        nc.vector.tensor_tensor(out=o[:], in0=o[:],
                                                in1=b2_sb[:], op=OP.add)
                        nc.sync.dma_start(out=out[r0:r0 + wsz, :],
                                          in_=o[:wsz, :])

        edge_phase(1, h_ext, ad1t, REC1, 4)
        nc.gpsimd.collective_compute("AllGather", OP.bypass, replica_groups=rg,
                                     ins=[h2sh[:]], outs=[h2full[:]])
        nc.gpsimd.collective_compute("AllGather", OP.bypass, replica_groups=rg,
                                     ins=[ad2sh[:]], outs=[ad2full[:]])
        edge_phase(2, h2full, ad2full, REC2, 1)

    return nc


# ===================== host-side preparation =====================

def make_weight_inputs(W1, a_src1, a_dst1, b1, W2, a_src2, a_dst2, b2):
    """Precompute permuted/extended weights. (c,h)-interleave: col c*4+h."""
    W1, W2 = np.asarray(W1, np.float32), np.asarray(W2, np.float32)
    a_src1, a_dst1 = np.asarray(a_src1, np.float32), np.asarray(a_dst1, np.float32)
    a_src2, a_dst2 = np.asarray(a_src2, np.float32), np.asarray(a_dst2, np.float32)
    b1, b2 = np.asarray(b1, np.float32), np.asarray(b2, np.float32)

    perm = np.empty(128, np.int64)        # perm[c*4+h] = h*32+c
    for c in range(HID):
        for h in range(HEADS):
            perm[c * 4 + h] = h * HID + c
    W1p = W1[:, perm]                               # [128, 128]
    ws1 = np.stack([W1[:, h * HID:(h + 1) * HID] @ a_src1[h] for h in range(HEADS)], 1)
    wd1 = np.stack([W1[:, h * HID:(h + 1) * HID] @ a_dst1[h] for h in range(HEADS)], 1)
    w1ext = np.concatenate([W1p, ws1, wd1], axis=1).astype(np.float32)  # [128,136]

    W2p = W2[perm, :]                               # [128, 32]
    ws2 = W2p @ a_src2[0][:, None]                  # [128, 1]
    wd2 = W2p @ a_dst2[0][:, None]
    w2ext = np.concatenate([W2p, ws2, wd2], axis=1).astype(np.float32)  # [128,34]

    b1p = b1[perm]
    b1r = np.broadcast_to(b1p, (128, 128)).copy().astype(np.float32)
    b2r = np.broadcast_to(b2, (128, 32)).copy().astype(np.float32)
    iotar = np.broadcast_to(np.arange(128, dtype=np.float32), (128, 128)).copy()
    identr = np.eye(128, dtype=np.float32)
    onesr = np.ones((128, 1), np.float32)
    return dict(w1ext=w1ext, w2ext=w2ext, b1r=b1r, b2r=b2r, iotar=iotar,
                identr=identr, onesr=onesr)


def make_edge_inputs(cfg: Cfg, edge_index):
    """Per-core padded window metadata. Returns list of dicts per core."""
    N, E = cfg.N, edge_index.shape[1]
    src = np.concatenate([edge_index[0], np.arange(N)]).astype(np.int64)
    dst = np.concatenate([edge_index[1], np.arange(N)]).astype(np.int64)
    order = np.argsort(dst, kind="stable")
    src, dst = src[order], dst[order]

    c = dst // cfg.npc
    loc = dst - c * cfg.npc
    wv = loc // cfg.WIN
    rel = loc - wv * cfg.WIN
    gw = c * cfg.nwin + wv
    ngw = cfg.n_cores * cfg.nwin
    counts = np.bincount(gw, minlength=ngw)
    tmax_needed = int(np.ceil(counts.max() / 128))
    assert tmax_needed <= cfg.TMAX, f"TMAX {cfg.TMAX} < needed {tmax_needed}"
    starts = np.zeros(ngw, np.int64)
    starts[1:] = np.cumsum(counts)[:-1]
    pos = np.arange(len(src)) - starts[gw]
    flat = gw * (cfg.TMAX * 128) + pos
    tot = ngw * cfg.TMAX * 128
    msrc = np.zeros(tot, np.int32)
    msrc[flat] = src
    mrel = np.full(tot, -1.0, np.float32)
    mrel[flat] = rel

    per_core = []
    for cc in range(cfg.n_cores):
        s = cc * cfg.nwin * cfg.TMAX * 128
        e = (cc + 1) * cfg.nwin * cfg.TMAX * 128
        wr = (cc * cfg.npc
              + np.arange(cfg.nwin * 128, dtype=np.int64)).clip(max=N - 1)
        per_core.append(dict(
            msrc=msrc[s:e, None].copy(),
            mrel=mrel[s:e, None].copy(),
            wrows=wr.astype(np.int32)[:, None],
        ))
    return per_core


def make_x_input(cfg: Cfg, x):
    x = np.asarray(x, np.float32)
    xp = np.zeros((cfg.Npad, IN_CH), np.float32)
    xp[:cfg.N] = x
    return np.ascontiguousarray(xp.T)


# ===================== SPMD runner =====================
import time
import numpy as np
import jax
from jax.sharding import Mesh, PartitionSpec
from jax.experimental.shard_map import shard_map

import concourse.bass as bass
from concourse import bass2jax


class SpmdRunner:
    def __init__(self, nc: bass.Bass, n_cores: int = 8):
        bass2jax.install_neuronx_cc_hook()
        if not nc.is_finalized():
            nc.finalize()
        self.nc = nc
        self.n_cores = n_cores

        in_names, out_names, out_avals, zero_outs = [], [], [], []
        partition_name = nc.partition_id_tensor.name if nc.partition_id_tensor else None
        import concourse.mybir as mybir
        for alloc in nc.m.functions[0].allocations:
            if not isinstance(alloc, mybir.MemoryLocationSet):
                continue
            name = alloc.memorylocations[0].name
            if alloc.kind == "ExternalInput":
                if name != partition_name:
                    in_names.append(name)
            elif alloc.kind == "ExternalOutput":
                out_names.append(name)
                shape = tuple(alloc.tensor_shape)
                dtype = mybir.dt.np(alloc.dtype)
                out_avals.append(jax.core.ShapedArray(shape, dtype))
                zero_outs.append(np.zeros(shape, dtype))
        self.in_names = list(in_names)
        self.out_names = out_names
        self.out_avals = out_avals
        self.zero_outs = zero_outs
        n_params = len(in_names)
        n_outs = len(out_avals)
        all_in_names = in_names + out_names
        if partition_name is not None:
            all_in_names.append(partition_name)
        self.partition_name = partition_name

        donate = tuple(range(n_params, n_params + n_outs))

        def _body(*args):
            operands = list(args)
            if partition_name is not None:
                operands.append(bass2jax.partition_id_tensor())
            outs = bass2jax._bass_exec_p.bind(
                *operands,
                out_avals=tuple(out_avals),
                in_names=tuple(all_in_names),
                out_names=tuple(out_names),
                lowering_input_output_aliases=(),
                sim_require_finite=True,
                sim_require_nnan=True,
                nc=nc,
            )
            return tuple(outs)

        devices = jax.devices()[:n_cores]
        self.mesh = Mesh(np.asarray(devices), ("core",))
        in_specs = (PartitionSpec("core"),) * (n_params + n_outs)
        out_specs = (PartitionSpec("core"),) * len(out_names)
        self.fn = jax.jit(
            shard_map(_body, mesh=self.mesh, in_specs=in_specs, out_specs=out_specs,
                      check_rep=False),
            donate_argnums=donate, keep_unused=True,
        )

    def put_inputs(self, in_maps: list[dict[str, np.ndarray]]):
        """Concat per-core inputs along axis 0 and device_put once."""
        n = self.n_cores
        concat_in = [
            np.ascontiguousarray(
                np.concatenate([np.asarray(in_maps[c][name]) for c in range(n)], axis=0))
            for name in self.in_names
        ]
        sharding = jax.sharding.NamedSharding(self.mesh, PartitionSpec("core"))
        return [jax.device_put(a, sharding) for a in concat_in]

    def run(self, dev_inputs):
        n = self.n_cores
        zeros = [np.zeros((n * z.shape[0], *z.shape[1:]), z.dtype) for z in self.zero_outs]
        out = self.fn(*dev_inputs, *zeros)
        jax.block_until_ready(out)
        return out

    def run_timed(self, dev_inputs, iters=5):
        # warmup (includes compile)
        out = self.run(dev_inputs)
        times = []
        for _ in range(iters):
            t0 = time.perf_counter()
            out = self.run(dev_inputs)
            t1 = time.perf_counter()
            times.append(t1 - t0)
        return out, times

    def results(self, out_arrs):
        n = self.n_cores
        return [
            {name: np.asarray(out_arrs[i]).reshape(n, *self.out_avals[i].shape)[c]
             for i, name in enumerate(self.out_names)}
            for c in range(n)
        ]


# ===================== public entry =====================

N_FULL = 100000
_CACHE = {}


def _get(cfg_key="full"):
    if cfg_key not in _CACHE:
        cfg = Cfg(N_FULL, n_cores=8, TMAX=19)
        nc = build_gat(cfg)
        r = SpmdRunner(nc, cfg.n_cores)
        _CACHE[cfg_key] = (cfg, r)
    return _CACHE[cfg_key]


def kernel(x, edge_index, W1, a_src1, a_dst1, b1, W2, a_src2, a_dst2, b2):
    cfg, r = _get()
    ei = np.asarray(edge_index).astype(np.int64)
    common = make_weight_inputs(W1, a_src1, a_dst1, b1, W2, a_src2, a_dst2, b2)
    common["xT"] = make_x_input(cfg, np.asarray(x, np.float32))
    per_core = make_edge_inputs(cfg, ei)
    in_maps = [dict(common, **pc) for pc in per_core]
    dev = r.put_inputs(in_maps)
    res = r.results(r.run(dev))
    out = np.concatenate([res[c]["out"] for c in range(cfg.n_cores)], axis=0)
    return out.astype(np.float32)


# revision 2
# speedup vs baseline: 2.2913x; 2.2913x over previous
"""Self-contained GAT-2-layer kernel for Trainium2 (8 NeuronCores, SPMD).

kernel(**inputs) takes the FULL unsharded inputs of nn_GAT_13400297964010
(x [100000,128] f32, edge_index [2,1600000] int, weights/biases) and
returns the full [100000, 32] f32 output, computed on 8 TRN2 cores.
"""
"""GAT 2-layer Bass kernel for TRN2, 8-core SPMD.

Sharding: nodes split into 8 contiguous dst-ranges (one per core); edges
(incl. self-loops) sorted by dst; each core owns all edges into its range,
grouped into 128-node dst windows, padded to TMAX 128-edge tiles/window.

Per core:
  Phase A: h_ext = x @ W1ext for ALL nodes -> DRAM [Npad, 136]
           (cols 0:128 h in (c,h)-interleave, 128:132 as1, 132:136 ad1;
           ad1 also copied to compact table).
  Phase B (L1): per window, accumulate psum[m, 136] over tiles:
           G = h_ext[src] (indirect gather), ind_em = (iota == dst_rel),
           ind_me = transpose(ind_em), ad_e = ind_me.T @ ad_win,
           w = exp(lrelu(as+ad)), F = G*w, psum += ind_em.T @ [F|junk|w].
  Flush:   h1 = numer/denom + b1, relu; h2 = h1T.T @ W2ext -> h2 shard
           (+ ad2 shard).
  AllGather shards -> full h2_ext / ad2 tables.
  Phase C (L2): same pipeline on 36-col records, 1 head -> out rows.
"""
import numpy as np
from contextlib import ExitStack

import concourse.bass as bass
import concourse.bacc as bacc
import concourse.mybir as mybir
from concourse.tile import TileContext

F32 = mybir.dt.float32
I32 = mybir.dt.int32
AF = mybir.ActivationFunctionType
OP = mybir.AluOpType

HEADS, HID, OUT_CH, IN_CH = 4, 32, 32, 128
REC1 = 136
REC2 = 36
NEG = 0.2


class Cfg:
    def __init__(self, N, n_cores=8, TMAX=19, WIN=128):
        self.N = N
        self.n_cores = n_cores
        self.npc = N // n_cores
        assert self.npc * n_cores == N
        self.WIN = WIN
        self.nwin = (self.npc + WIN - 1) // WIN
        self.win_sizes = [min(WIN, self.npc - w * WIN) for w in range(self.nwin)]
        self.TMAX = TMAX
        self.Npad = ((N + 127) // 128) * 128
        self.ntiles_a = self.Npad // 128
        self.S = self.nwin * self.TMAX * 128


def build_gat(cfg: Cfg):
    nc = bacc.Bacc("TRN2", target_bir_lowering=False, debug=False,
                   num_devices=cfg.n_cores)

    xT = nc.declare_dram_parameter("xT", [IN_CH, cfg.Npad], F32, isOutput=False)
    w1ext = nc.declare_dram_parameter("w1ext", [IN_CH, REC1], F32, isOutput=False)
    w2ext = nc.declare_dram_parameter("w2ext", [128, 34], F32, isOutput=False)
    b1r = nc.declare_dram_parameter("b1r", [128, 128], F32, isOutput=False)
    b2r = nc.declare_dram_parameter("b2r", [128, 32], F32, isOutput=False)
    iotar = nc.declare_dram_parameter("iotar", [128, 128], F32, isOutput=False)
    identr = nc.declare_dram_parameter("identr", [128, 128], F32, isOutput=False)
    onesr = nc.declare_dram_parameter("onesr", [128, 1], F32, isOutput=False)
    msrc = nc.declare_dram_parameter("msrc", [cfg.S, 1], I32, isOutput=False)
    mrel = nc.declare_dram_parameter("mrel", [cfg.S, 1], F32, isOutput=False)
    wrows = nc.declare_dram_parameter("wrows", [cfg.nwin * 128, 1], I32, isOutput=False)
    out = nc.declare_dram_parameter("out", [cfg.npc, 32], F32, isOutput=True)

    h_ext = nc.dram_tensor("h_ext", [cfg.Npad, REC1], F32)
    ad1t = nc.dram_tensor("ad1t", [cfg.Npad, 4], F32)
    h2sh = nc.dram_tensor("h2sh", [cfg.npc, REC2], F32)
    ad2sh = nc.dram_tensor("ad2sh", [cfg.npc, 4], F32)
    h2full = nc.dram_tensor("h2full", [cfg.N, REC2], F32, addr_space="Shared")
    ad2full = nc.dram_tensor("ad2full", [cfg.N, 4], F32, addr_space="Shared")

    rg = [list(range(cfg.n_cores))]

    with TileContext(nc) as tc, ExitStack() as top:
        cpool = top.enter_context(tc.tile_pool(name="consts", bufs=1))

        def cload(shape, src, tag):
            t = cpool.tile(shape, F32, tag=tag)
            nc.sync.dma_start(out=t[:], in_=src[:])
            return t

        w1_sb = cload([IN_CH, REC1], w1ext, "w1c")
        w2_sb = cload([128, 34], w2ext, "w2c")
        b1_sb = cload([128, 128], b1r, "b1c")
        b2_sb = cload([128, 32], b2r, "b2c")
        iota_sb = cload([128, 128], iotar, "iotac")
        id_sb = cload([128, 128], identr, "identc")
        ones_sb = cload([128, 1], onesr, "onesc")
        wrows_sb = cpool.tile([128, cfg.nwin], I32)
        nc.sync.dma_start(out=wrows_sb[:],
                          in_=wrows[:, 0].rearrange("(w p) -> p w", p=128))

        # ---------------- Phase A ----------------
        with tc.tile_pool(name="pa_sb", bufs=4) as pa_sb, \
             tc.tile_pool(name="pa_ps", bufs=4, space="PSUM") as pa_ps:
            for t in range(cfg.ntiles_a):
                xt = pa_sb.tile([IN_CH, 128], F32, tag="xt")
                nc.sync.dma_start(out=xt[:], in_=xT[:, t * 128:(t + 1) * 128])
                hp = pa_ps.tile([128, REC1], F32, tag="hp")
                nc.tensor.matmul(out=hp[:], lhsT=xt[:], rhs=w1_sb[:],
                                 start=True, stop=True)
                hs = pa_sb.tile([128, REC1], F32, tag="hs")
                nc.vector.tensor_copy(out=hs[:], in_=hp[:])
                nc.sync.dma_start(out=h_ext[t * 128:(t + 1) * 128, :], in_=hs[:])
                nc.sync.dma_start(out=ad1t[t * 128:(t + 1) * 128, :],
                                  in_=hs[:, 132:136])

        # ---------------- edge phases ----------------
        def edge_phase(layer, table, ad_tab, rec, nheads):
            wrec = rec if layer == 1 else 34
            with tc.tile_pool(name=f"e{layer}m", bufs=3) as mpool, \
                 tc.tile_pool(name=f"e{layer}s", bufs=8) as spool, \
                 tc.tile_pool(name=f"e{layer}i", bufs=8) as ipool, \
                 tc.tile_pool(name=f"e{layer}f", bufs=2) as fpool, \
                 tc.tile_pool(name=f"e{layer}w", bufs=2, space="PSUM") as wps, \
                 tc.tile_pool(name=f"e{layer}t", bufs=2, space="PSUM") as tps, \
                 tc.tile_pool(name=f"e{layer}a", bufs=2, space="PSUM") as aps:
                for w in range(cfg.nwin):
                    wsz = cfg.win_sizes[w]
                    base = w * cfg.TMAX * 128
                    srcw = mpool.tile([128, cfg.TMAX], I32, tag="srcw")
                    nc.sync.dma_start(
                        out=srcw[:],
                        in_=msrc[base:base + cfg.TMAX * 128, 0]
                        .rearrange("(t p) -> p t", p=128))
                    relw = mpool.tile([128, cfg.TMAX], F32, tag="relw")
                    nc.sync.dma_start(
                        out=relw[:],
                        in_=mrel[base:base + cfg.TMAX * 128, 0]
                        .rearrange("(t p) -> p t", p=128))
                    adw = mpool.tile([128, 4], F32, tag="adw")
                    nc.gpsimd.indirect_dma_start(
                        out=adw[:, :nheads] if nheads == 4 else adw[:, 0:nheads],
                        out_offset=None, in_=ad_tab[:],
                        in_offset=bass.IndirectOffsetOnAxis(
                            ap=wrows_sb[:, w:w + 1], axis=0))

                    winp = wps.tile([128, wrec], F32, tag="winp")
                    for t in range(cfg.TMAX):
                        g = spool.tile([128, rec], F32, tag="g")
                        nc.gpsimd.indirect_dma_start(
                            out=g[:], out_offset=None, in_=table[:],
                            in_offset=bass.IndirectOffsetOnAxis(
                                ap=srcw[:, t:t + 1], axis=0))
                        ind_em = ipool.tile([128, 128], F32, tag="ind_em")
                        nc.vector.tensor_scalar(
                            out=ind_em[:], in0=iota_sb[:],
                            scalar1=relw[:, t:t + 1], scalar2=None,
                            op0=OP.is_equal)
                        ime_ps = tps.tile([128, 128], F32, tag="ime")
                        nc.tensor.transpose(out=ime_ps[:], in_=ind_em[:],
                                            identity=id_sb[:])
                        ind_me = ipool.tile([128, 128], F32, tag="ind_me")
                        nc.scalar.activation(out=ind_me[:], in_=ime_ps[:],
                                             func=AF.Copy)
                        adp = aps.tile([128, 4], F32, tag="adp")
                        nc.tensor.matmul(out=adp[:, :nheads], lhsT=ind_me[:],
                                         rhs=adw[:, :nheads], start=True, stop=True)
                        if layer == 1:
                            asl, wsl = g[:, 128:132], g[:, 132:136]
                        else:
                            asl, wsl = g[:, 32:33], g[:, 34:35]
                        esum = spool.tile([128, 4], F32, tag="esum")
                        nc.vector.tensor_tensor(out=esum[:, :nheads], in0=asl,
                                                in1=adp[:, :nheads], op=OP.add)
                        elr = spool.tile([128, 4], F32, tag="elr")
                        nc.vector.tensor_scalar(out=elr[:, :nheads],
                                                in0=esum[:, :nheads],
                                                scalar1=NEG, scalar2=None,
                                                op0=OP.mult)
                        nc.vector.tensor_tensor(out=elr[:, :nheads],
                                                in0=esum[:, :nheads],
                                                in1=elr[:, :nheads],
                                                op=OP.max)
                        nc.scalar.activation(out=wsl, in_=elr[:, :nheads],
                                             func=AF.Exp)
                        if layer == 1:
                            wap = bass.AP(wsl.tensor, wsl.offset,
                                          [wsl.ap[0], [0, 32], [1, 4]])
                            nc.vector.tensor_tensor(out=g[:, 0:128],
                                                    in0=g[:, 0:128], in1=wap,
                                                    op=OP.mult)
                        else:
                            nc.vector.tensor_scalar(out=g[:, 0:34],
                                                    in0=g[:, 0:34], scalar1=wsl,
                                                    scalar2=None, op0=OP.mult)
                        nc.tensor.matmul(out=winp[:], lhsT=ind_em[:],
                                         rhs=g[:, 0:wrec],
                                         start=(t == 0),
                                         stop=(t == cfg.TMAX - 1))

                    r0 = w * 128
                    if layer == 1:
                        denr = fpool.tile([128, 4], F32, tag="denr")
                        nc.vector.reciprocal(out=denr[:], in_=winp[:, 132:136])
                        h1 = fpool.tile([128, 128], F32, tag="h1")
                        pv = winp[:, 0:128].rearrange("p (c h) -> p h c", h=4)
                        hv = h1[:].rearrange("p (c h) -> p h c", h=4)
                        for hh in range(4):
                            nc.vector.tensor_scalar(
                                out=hv[:, hh, :], in0=pv[:, hh, :],
                                scalar1=denr[:, hh:hh + 1], scalar2=None,
                                op0=OP.mult)
                        nc.vector.tensor_tensor(out=h1[:], in0=h1[:],
                                                in1=b1_sb[:], op=OP.add)
                        nc.scalar.activation(out=h1[:], in_=h1[:], func=AF.Relu)
                        h1tp = tps.tile([128, 128], F32, tag="ime")
                        nc.tensor.transpose(out=h1tp[:], in_=h1[:],
                                            identity=id_sb[:])
                        h1t = fpool.tile([128, 128], F32, tag="h1t")
                        nc.scalar.activation(out=h1t[:], in_=h1tp[:], func=AF.Copy)
                        h2p = aps.tile([128, 34], F32, tag="adp")
                        nc.tensor.matmul(out=h2p[:], lhsT=h1t[:], rhs=w2_sb[:],
                                         start=True, stop=True)
                        h2s = fpool.tile([128, 34], F32, tag="h2s")
                        nc.vector.tensor_copy(out=h2s[:], in_=h2p[:])
                        nc.sync.dma_start(out=h2sh[r0:r0 + wsz, 0:33],
                                          in_=h2s[:wsz, 0:33])
                        nc.sync.dma_start(out=h2sh[r0:r0 + wsz, 33:34],
                                          in_=ones_sb[:wsz, :])
                        nc.sync.dma_start(out=ad2sh[r0:r0 + wsz, 0:1],
                                          in_=h2s[:wsz, 33:34])
                    else:
                        denr = fpool.tile([128, 1], F32, tag="denr2")
                        nc.vector.reciprocal(out=denr[:], in_=winp[:, 33:34])
                        o = fpool.tile([128, 32], F32, tag="o")
                        nc.vector.tensor_scalar(out=o[:], in0=winp[:, 0:32],
                                                scalar1=denr[:], scalar2=None,
                                                op0=OP.mult)
                        nc.vector.tensor_tensor(out=o[:], in0=o[:],
                                                in1=b2_sb[:], op=OP.add)
                        nc.sync.dma_start(out=out[r0:r0 + wsz, :],
                                          in_=o[:wsz, :])

        edge_phase(1, h_ext, ad1t, REC1, 4)
        nc.gpsimd.collective_compute("AllGather", OP.bypass, replica_groups=rg,
                                     ins=[h2sh[:]], outs=[h2full[:]])
        nc.gpsimd.collective_compute("AllGather", OP.bypass, replica_groups=rg,
                                     ins=[ad2sh[:]], outs=[ad2full[:]])
        edge_phase(2, h2full, ad2full, REC2, 1)

    return nc


# ===================== host-side preparation =====================

def make_weight_inputs(W1, a_src1, a_dst1, b1, W2, a_src2, a_dst2, b2):
    """Precompute permuted/extended weights. (c,h)-interleave: col c*4+h."""
    W1, W2 = np.asarray(W1, np.float32), np.asarray(W2, np.float32)
    a_src1, a_dst1 = np.asarray(a_src1, np.float32), np.asarray(a_dst1, np.float32)
    a_src2, a_dst2 = np.asarray(a_src2, np.float32), np.asarray(a_dst2, np.float32)
    b1, b2 = np.asarray(b1, np.float32), np.asarray(b2, np.float32)

    perm = np.empty(128, np.int64)        # perm[c*4+h] = h*32+c
    for c in range(HID):
        for h in range(HEADS):
            perm[c * 4 + h] = h * HID + c
    W1p = W1[:, perm]                               # [128, 128]
    ws1 = np.stack([W1[:, h * HID:(h + 1) * HID] @ a_src1[h] for h in range(HEADS)], 1)
    wd1 = np.stack([W1[:, h * HID:(h + 1) * HID] @ a_dst1[h] for h in range(HEADS)], 1)
    w1ext = np.concatenate([W1p, ws1, wd1], axis=1).astype(np.float32)  # [128,136]

    W2p = W2[perm, :]                               # [128, 32]
    ws2 = W2p @ a_src2[0][:, None]                  # [128, 1]
    wd2 = W2p @ a_dst2[0][:, None]
    w2ext = np.concatenate([W2p, ws2, wd2], axis=1).astype(np.float32)  # [128,34]

    b1p = b1[perm]
    b1r = np.broadcast_to(b1p, (128, 128)).copy().astype(np.float32)
    b2r = np.broadcast_to(b2, (128, 32)).copy().astype(np.float32)
    iotar = np.broadcast_to(np.arange(128, dtype=np.float32), (128, 128)).copy()
    identr = np.eye(128, dtype=np.float32)
    onesr = np.ones((128, 1), np.float32)
    return dict(w1ext=w1ext, w2ext=w2ext, b1r=b1r, b2r=b2r, iotar=iotar,
                identr=identr, onesr=onesr)


def make_edge_inputs(cfg: Cfg, edge_index):
    """Per-core padded window metadata. Returns list of dicts per core."""
    N, E = cfg.N, edge_index.shape[1]
    src = np.concatenate([edge_index[0], np.arange(N)]).astype(np.int64)
    dst = np.concatenate([edge_index[1], np.arange(N)]).astype(np.int64)
    order = np.argsort(dst, kind="stable")
    src, dst = src[order], dst[order]

    c = dst // cfg.npc
    loc = dst - c * cfg.npc
    wv = loc // cfg.WIN
    rel = loc - wv * cfg.WIN
    gw = c * cfg.nwin + wv
    ngw = cfg.n_cores * cfg.nwin
    counts = np.bincount(gw, minlength=ngw)
    tmax_needed = int(np.ceil(counts.max() / 128))
    assert tmax_needed <= cfg.TMAX, f"TMAX {cfg.TMAX} < needed {tmax_needed}"
    starts = np.zeros(ngw, np.int64)
    starts[1:] = np.cumsum(counts)[:-1]
    pos = np.arange(len(src)) - starts[gw]
    flat = gw * (cfg.TMAX * 128) + pos
    tot = ngw * cfg.TMAX * 128
    msrc = np.zeros(tot, np.int32)
    msrc[flat] = src
    mrel = np.full(tot, -1.0, np.float32)
    mrel[flat] = rel

    per_core = []
    for cc in range(cfg.n_cores):
        s = cc * cfg.nwin * cfg.TMAX * 128
        e = (cc + 1) * cfg.nwin * cfg.TMAX * 128
        wr = (cc * cfg.npc
              + np.arange(cfg.nwin * 128, dtype=np.int64)).clip(max=N - 1)
        per_core.append(dict(
            msrc=msrc[s:e, None].copy(),
            mrel=mrel[s:e, None].copy(),
            wrows=wr.astype(np.int32)[:, None],
        ))
    return per_core


def make_x_input(cfg: Cfg, x):
    x = np.asarray(x, np.float32)
    xp = np.zeros((cfg.Npad, IN_CH), np.float32)
    xp[:cfg.N] = x
    return np.ascontiguousarray(xp.T)


# ===================== SPMD runner =====================
import time
import numpy as np
import jax
from jax.sharding import Mesh, PartitionSpec
from jax.experimental.shard_map import shard_map

import concourse.bass as bass
from concourse import bass2jax


class SpmdRunner:
    def __init__(self, nc: bass.Bass, n_cores: int = 8):
        bass2jax.install_neuronx_cc_hook()
        if not nc.is_finalized():
            nc.finalize()
        self.nc = nc
        self.n_cores = n_cores

        in_names, out_names, out_avals, zero_outs = [], [], [], []
        partition_name = nc.partition_id_tensor.name if nc.partition_id_tensor else None
        import concourse.mybir as mybir
        for alloc in nc.m.functions[0].allocations:
            if not isinstance(alloc, mybir.MemoryLocationSet):
                continue
            name = alloc.memorylocations[0].name
            if alloc.kind == "ExternalInput":
                if name != partition_name:
                    in_names.append(name)
            elif alloc.kind == "ExternalOutput":
                out_names.append(name)
                shape = tuple(alloc.tensor_shape)
                dtype = mybir.dt.np(alloc.dtype)
                out_avals.append(jax.core.ShapedArray(shape, dtype))
                zero_outs.append(np.zeros(shape, dtype))
        self.in_names = list(in_names)
        self.out_names = out_names
        self.out_avals = out_avals
        self.zero_outs = zero_outs
        n_params = len(in_names)
        n_outs = len(out_avals)
        all_in_names = in_names + out_names
        if partition_name is not None:
            all_in_names.append(partition_name)
        self.partition_name = partition_name

        donate = tuple(range(n_params, n_params + n_outs))

        def _body(*args):
            operands = list(args)
            if partition_name is not None:
                operands.append(bass2jax.partition_id_tensor())
            outs = bass2jax._bass_exec_p.bind(
                *operands,
                out_avals=tuple(out_avals),
                in_names=tuple(all_in_names),
                out_names=tuple(out_names),
                lowering_input_output_aliases=(),
                sim_require_finite=True,
                sim_require_nnan=True,
                nc=nc,
            )
            return tuple(outs)

        devices = jax.devices()[:n_cores]
        self.mesh = Mesh(np.asarray(devices), ("core",))
        in_specs = (PartitionSpec("core"),) * (n_params + n_outs)
        out_specs = (PartitionSpec("core"),) * len(out_names)
        self.fn = jax.jit(
            shard_map(_body, mesh=self.mesh, in_specs=in_specs, out_specs=out_specs,
                      check_rep=False),
            donate_argnums=donate, keep_unused=True,
        )

    def put_inputs(self, in_maps: list[dict[str, np.ndarray]]):
        """Concat per-core inputs along axis 0 and device_put once."""
        n = self.n_cores
        concat_in = [
            np.ascontiguousarray(
                np.concatenate([np.asarray(in_maps[c][name]) for c in range(n)], axis=0))
            for name in self.in_names
        ]
        sharding = jax.sharding.NamedSharding(self.mesh, PartitionSpec("core"))
        return [jax.device_put(a, sharding) for a in concat_in]

    def _put_zeros(self):
        n = self.n_cores
        sharding = jax.sharding.NamedSharding(
            self.mesh, PartitionSpec("core"))
        return [jax.device_put(
            np.zeros((n * z.shape[0], *z.shape[1:]), z.dtype), sharding)
            for z in self.zero_outs]

    def run(self, dev_inputs, zeros=None):
        if zeros is None:
            zeros = self._put_zeros()
        out = self.fn(*dev_inputs, *zeros)
        jax.block_until_ready(out)
        return out

    def run_timed(self, dev_inputs, iters=5):
        # warmup (includes compile)
        out = self.run(dev_inputs)
        times = []
        for _ in range(iters):
            zeros = self._put_zeros()
            jax.block_until_ready(zeros)
            t0 = time.perf_counter()
            out = self.fn(*dev_inputs, *zeros)
            jax.block_until_ready(out)
            t1 = time.perf_counter()
            times.append(t1 - t0)
        return out, times

    def results(self, out_arrs):
        n = self.n_cores
        return [
            {name: np.asarray(out_arrs[i]).reshape(n, *self.out_avals[i].shape)[c]
             for i, name in enumerate(self.out_names)}
            for c in range(n)
        ]


# ===================== public entry =====================

N_FULL = 100000
_CACHE = {}


def _get(cfg_key="full"):
    if cfg_key not in _CACHE:
        cfg = Cfg(N_FULL, n_cores=8, TMAX=19)
        nc = build_gat(cfg)
        r = SpmdRunner(nc, cfg.n_cores)
        _CACHE[cfg_key] = (cfg, r)
    return _CACHE[cfg_key]


def kernel(x, edge_index, W1, a_src1, a_dst1, b1, W2, a_src2, a_dst2, b2):
    cfg, r = _get()
    ei = np.asarray(edge_index).astype(np.int64)
    common = make_weight_inputs(W1, a_src1, a_dst1, b1, W2, a_src2, a_dst2, b2)
    common["xT"] = make_x_input(cfg, np.asarray(x, np.float32))
    per_core = make_edge_inputs(cfg, ei)
    in_maps = [dict(common, **pc) for pc in per_core]
    dev = r.put_inputs(in_maps)
    res = r.results(r.run(dev))
    out = np.concatenate([res[c]["out"] for c in range(cfg.n_cores)], axis=0)
    return out.astype(np.float32)


# revision 3
# speedup vs baseline: 11.4352x; 4.9906x over previous
"""Self-contained 2-layer GAT kernel for Trainium2 (8 NeuronCores, SPMD).

kernel(**inputs) takes the FULL unsharded inputs (x [100000,128] f32,
edge_index [2,1600000] int, weights/biases) and returns the full
[100000, 32] f32 output, computed on 8 TRN2 cores via Bass.
"""
"""GAT 2-layer Bass kernel for TRN2, 8-core SPMD.

Sharding: nodes split into 8 contiguous dst-ranges (one per core); edges
(incl. self-loops) sorted by dst; each core owns all edges into its range,
grouped into 128-node dst windows, padded to TMAX 128-edge tiles/window.

Per core:
  Phase A: h_ext = x @ W1ext for ALL nodes -> DRAM [Npad, 136]
           (cols 0:128 h in (c,h)-interleave, 128:132 as1, 132:136 ad1;
           ad1 also copied to compact table).
  Phase B (L1): per window, accumulate psum[m, 136] over tiles:
           G = h_ext[src] (indirect gather), ind_em = (iota == dst_rel),
           ind_me = transpose(ind_em), ad_e = ind_me.T @ ad_win,
           w = exp(lrelu(as+ad)), F = G*w, psum += ind_em.T @ [F|junk|w].
  Flush:   h1 = numer/denom + b1, relu; h2 = h1T.T @ W2ext -> h2 shard
           (+ ad2 shard).
  AllGather shards -> full h2_ext / ad2 tables.
  Phase C (L2): same pipeline on 36-col records, 1 head -> out rows.
"""
import numpy as np
from contextlib import ExitStack

import concourse.bass as bass
import concourse.bacc as bacc
import concourse.mybir as mybir
from concourse.tile import TileContext

F32 = mybir.dt.float32
I32 = mybir.dt.int32
AF = mybir.ActivationFunctionType
OP = mybir.AluOpType

HEADS, HID, OUT_CH, IN_CH = 4, 32, 32, 128
REC1 = 136
REC2 = 36
NEG = 0.2


class Cfg:
    def __init__(self, N, n_cores=8, TMAX=19, WIN=128):
        self.N = N
        self.n_cores = n_cores
        self.npc = N // n_cores
        assert self.npc * n_cores == N
        self.WIN = WIN
        self.nwin = (self.npc + WIN - 1) // WIN
        self.win_sizes = [min(WIN, self.npc - w * WIN) for w in range(self.nwin)]
        self.TMAX = TMAX
        self.Npad = ((N + 127) // 128) * 128
        self.ntiles_a = self.Npad // 128
        self.S = self.nwin * self.TMAX * 128


def build_gat(cfg: Cfg):
    nc = bacc.Bacc("TRN2", target_bir_lowering=False, debug=False,
                   num_devices=cfg.n_cores)

    xT = nc.declare_dram_parameter("xT", [IN_CH, cfg.Npad], F32, isOutput=False)
    w1ext = nc.declare_dram_parameter("w1ext", [IN_CH, REC1], F32, isOutput=False)
    w2ext = nc.declare_dram_parameter("w2ext", [128, 34], F32, isOutput=False)
    b1r = nc.declare_dram_parameter("b1r", [128, 128], F32, isOutput=False)
    b2r = nc.declare_dram_parameter("b2r", [128, 32], F32, isOutput=False)
    iotar = nc.declare_dram_parameter("iotar", [128, 128], F32, isOutput=False)
    identr = nc.declare_dram_parameter("identr", [128, 128], F32, isOutput=False)
    onesr = nc.declare_dram_parameter("onesr", [128, 1], F32, isOutput=False)
    msrc = nc.declare_dram_parameter("msrc", [cfg.S, 1], I32, isOutput=False)
    mrel = nc.declare_dram_parameter("mrel", [cfg.S, 1], F32, isOutput=False)
    wrows = nc.declare_dram_parameter("wrows", [cfg.nwin * 128, 1], I32, isOutput=False)
    out = nc.declare_dram_parameter("out", [cfg.npc, 32], F32, isOutput=True)

    h_ext = nc.dram_tensor("h_ext", [cfg.Npad, REC1], F32)
    ad1t = nc.dram_tensor("ad1t", [cfg.Npad, 4], F32)
    h2sh = nc.dram_tensor("h2sh", [cfg.npc, REC2], F32)
    ad2sh = nc.dram_tensor("ad2sh", [cfg.npc, 4], F32)
    h2full = nc.dram_tensor("h2full", [cfg.N, REC2], F32, addr_space="Shared")
    ad2full = nc.dram_tensor("ad2full", [cfg.N, 4], F32, addr_space="Shared")

    rg = [list(range(cfg.n_cores))]

    with TileContext(nc) as tc, ExitStack() as top:
        cpool = top.enter_context(tc.tile_pool(name="consts", bufs=1))

        def cload(shape, src, tag):
            t = cpool.tile(shape, F32, tag=tag)
            nc.sync.dma_start(out=t[:], in_=src[:])
            return t

        w1_sb = cload([IN_CH, REC1], w1ext, "w1c")
        w2_sb = cload([128, 34], w2ext, "w2c")
        b1_sb = cload([128, 128], b1r, "b1c")
        b2_sb = cload([128, 32], b2r, "b2c")
        iota_sb = cload([128, 128], iotar, "iotac")
        id_sb = cload([128, 128], identr, "identc")
        ones_sb = cload([128, 1], onesr, "onesc")
        wrows_sb = cpool.tile([128, cfg.nwin], I32)
        nc.sync.dma_start(out=wrows_sb[:],
                          in_=wrows[:, 0].rearrange("(w p) -> p w", p=128))

        # ---------------- Phase A ----------------
        with tc.tile_pool(name="pa_sb", bufs=4) as pa_sb, \
             tc.tile_pool(name="pa_ps", bufs=4, space="PSUM") as pa_ps:
            for t in range(cfg.ntiles_a):
                xt = pa_sb.tile([IN_CH, 128], F32, tag="xt")
                nc.sync.dma_start(out=xt[:], in_=xT[:, t * 128:(t + 1) * 128])
                hp = pa_ps.tile([128, REC1], F32, tag="hp")
                nc.tensor.matmul(out=hp[:], lhsT=xt[:], rhs=w1_sb[:],
                                 start=True, stop=True)
                hs = pa_sb.tile([128, REC1], F32, tag="hs")
                nc.vector.tensor_copy(out=hs[:], in_=hp[:])
                nc.sync.dma_start(out=h_ext[t * 128:(t + 1) * 128, :], in_=hs[:])
                nc.sync.dma_start(out=ad1t[t * 128:(t + 1) * 128, :],
                                  in_=hs[:, 132:136])

        # ---------------- edge phases ----------------
        def edge_phase(layer, table, ad_tab, rec, nheads):
            wrec = rec if layer == 1 else 34
            with tc.tile_pool(name=f"e{layer}m", bufs=3) as mpool, \
                 tc.tile_pool(name=f"e{layer}s", bufs=22) as spool, \
                 tc.tile_pool(name=f"e{layer}i", bufs=22) as ipool, \
                 tc.tile_pool(name=f"e{layer}f", bufs=2) as fpool, \
                 tc.tile_pool(name=f"e{layer}w", bufs=2, space="PSUM") as wps, \
                 tc.tile_pool(name=f"e{layer}t", bufs=3, space="PSUM") as tps, \
                 tc.tile_pool(name=f"e{layer}a", bufs=3, space="PSUM") as aps:
                for w in range(cfg.nwin):
                    wsz = cfg.win_sizes[w]
                    base = w * cfg.TMAX * 128
                    srcw = mpool.tile([128, cfg.TMAX], I32, tag="srcw")
                    nc.sync.dma_start(
                        out=srcw[:],
                        in_=msrc[base:base + cfg.TMAX * 128, 0]
                        .rearrange("(t p) -> p t", p=128))
                    relw = mpool.tile([128, cfg.TMAX], F32, tag="relw")
                    nc.sync.dma_start(
                        out=relw[:],
                        in_=mrel[base:base + cfg.TMAX * 128, 0]
                        .rearrange("(t p) -> p t", p=128))
                    adw = mpool.tile([128, 4], F32, tag="adw")
                    nc.gpsimd.indirect_dma_start(
                        out=adw[:, :nheads] if nheads == 4 else adw[:, 0:nheads],
                        out_offset=None, in_=ad_tab[:],
                        in_offset=bass.IndirectOffsetOnAxis(
                            ap=wrows_sb[:, w:w + 1], axis=0))

                    winp = wps.tile([128, wrec], F32, tag="winp")
                    gs, ems, mes = [], [], []
                    for t in range(cfg.TMAX):
                        g = spool.tile([128, rec], F32, tag="g")
                        nc.gpsimd.indirect_dma_start(
                            out=g[:], out_offset=None, in_=table[:],
                            in_offset=bass.IndirectOffsetOnAxis(
                                ap=srcw[:, t:t + 1], axis=0))
                        ind_em = ipool.tile([128, 128], F32, tag="ind_em")
                        nc.vector.tensor_scalar(
                            out=ind_em[:], in0=iota_sb[:],
                            scalar1=relw[:, t:t + 1], scalar2=None,
                            op0=OP.is_equal)
                        ime_ps = tps.tile([128, 128], F32, tag="ime")
                        nc.tensor.transpose(out=ime_ps[:], in_=ind_em[:],
                                            identity=id_sb[:])
                        ind_me = ipool.tile([128, 128], F32, tag="ind_me")
                        nc.scalar.activation(out=ind_me[:], in_=ime_ps[:],
                                             func=AF.Copy)
                        gs.append(g)
                        ems.append(ind_em)
                        mes.append(ind_me)
                    for t in range(cfg.TMAX):
                        g, ind_em, ind_me = gs[t], ems[t], mes[t]
                        adp = aps.tile([128, 4], F32, tag="adp")
                        nc.tensor.matmul(out=adp[:, :nheads], lhsT=ind_me[:],
                                         rhs=adw[:, :nheads], start=True, stop=True)
                        if layer == 1:
                            asl, wsl = g[:, 128:132], g[:, 132:136]
                        else:
                            asl, wsl = g[:, 32:33], g[:, 34:35]
                        esum = spool.tile([128, 4], F32, tag="esum")
                        nc.vector.tensor_tensor(out=esum[:, :nheads], in0=asl,
                                                in1=adp[:, :nheads], op=OP.add)
                        elr = spool.tile([128, 4], F32, tag="elr")
                        nc.vector.tensor_scalar(out=elr[:, :nheads],
                                                in0=esum[:, :nheads],
                                                scalar1=NEG, scalar2=None,
                                                op0=OP.mult)
                        nc.vector.tensor_tensor(out=elr[:, :nheads],
                                                in0=esum[:, :nheads],
                                                in1=elr[:, :nheads],
                                                op=OP.max)
                        nc.scalar.activation(out=wsl, in_=elr[:, :nheads],
                                             func=AF.Exp)
                        if layer == 1:
                            wap = bass.AP(wsl.tensor, wsl.offset,
                                          [wsl.ap[0], [0, 32], [1, 4]])
                            nc.vector.tensor_tensor(out=g[:, 0:128],
                                                    in0=g[:, 0:128], in1=wap,
                                                    op=OP.mult)
                        else:
                            nc.vector.tensor_scalar(out=g[:, 0:34],
                                                    in0=g[:, 0:34], scalar1=wsl,
                                                    scalar2=None, op0=OP.mult)
                        nc.tensor.matmul(out=winp[:], lhsT=ind_em[:],
                                         rhs=g[:, 0:wrec],
                                         start=(t == 0),
                                         stop=(t == cfg.TMAX - 1))

                    r0 = w * 128
                    if layer == 1:
                        denr = fpool.tile([128, 4], F32, tag="denr")
                        nc.vector.reciprocal(out=denr[:], in_=winp[:, 132:136])
                        h1 = fpool.tile([128, 128], F32, tag="h1")
                        pv = winp[:, 0:128].rearrange("p (c h) -> p h c", h=4)
                        hv = h1[:].rearrange("p (c h) -> p h c", h=4)
                        for hh in range(4):
                            nc.vector.tensor_scalar(
                                out=hv[:, hh, :], in0=pv[:, hh, :],
                                scalar1=denr[:, hh:hh + 1], scalar2=None,
                                op0=OP.mult)
                        nc.vector.tensor_tensor(out=h1[:], in0=h1[:],
                                                in1=b1_sb[:], op=OP.add)
                        nc.scalar.activation(out=h1[:], in_=h1[:], func=AF.Relu)
                        h1tp = tps.tile([128, 128], F32, tag="ime")
                        nc.tensor.transpose(out=h1tp[:], in_=h1[:],
                                            identity=id_sb[:])
                        h1t = fpool.tile([128, 128], F32, tag="h1t")
                        nc.scalar.activation(out=h1t[:], in_=h1tp[:], func=AF.Copy)
                        h2p = aps.tile([128, 34], F32, tag="adp")
                        nc.tensor.matmul(out=h2p[:], lhsT=h1t[:], rhs=w2_sb[:],
                                         start=True, stop=True)
                        h2s = fpool.tile([128, 34], F32, tag="h2s")
                        nc.vector.tensor_copy(out=h2s[:], in_=h2p[:])
                        nc.sync.dma_start(out=h2sh[r0:r0 + wsz, 0:33],
                                          in_=h2s[:wsz, 0:33])
                        nc.sync.dma_start(out=h2sh[r0:r0 + wsz, 33:34],
                                          in_=ones_sb[:wsz, :])
                        nc.sync.dma_start(out=ad2sh[r0:r0 + wsz, 0:1],
                                          in_=h2s[:wsz, 33:34])
                    else:
                        denr = fpool.tile([128, 1], F32, tag="denr2")
                        nc.vector.reciprocal(out=denr[:], in_=winp[:, 33:34])
                        o = fpool.tile([128, 32], F32, tag="o")
                        nc.vector.tensor_scalar(out=o[:], in0=winp[:, 0:32],
                                                scalar1=denr[:], scalar2=None,
                                                op0=OP.mult)
                        nc.vector.tensor_tensor(out=o[:], in0=o[:],
                                                in1=b2_sb[:], op=OP.add)
                        nc.sync.dma_start(out=out[r0:r0 + wsz, :],
                                          in_=o[:wsz, :])

        edge_phase(1, h_ext, ad1t, REC1, 4)
        nc.gpsimd.collective_compute("AllGather", OP.bypass, replica_groups=rg,
                                     ins=[h2sh[:]], outs=[h2full[:]])
        nc.gpsimd.collective_compute("AllGather", OP.bypass, replica_groups=rg,
                                     ins=[ad2sh[:]], outs=[ad2full[:]])
        edge_phase(2, h2full, ad2full, REC2, 1)

    return nc


# ===================== host-side preparation =====================

def make_weight_inputs(W1, a_src1, a_dst1, b1, W2, a_src2, a_dst2, b2):
    """Precompute permuted/extended weights. (c,h)-interleave: col c*4+h."""
    W1, W2 = np.asarray(W1, np.float32), np.asarray(W2, np.float32)
    a_src1, a_dst1 = np.asarray(a_src1, np.float32), np.asarray(a_dst1, np.float32)
    a_src2, a_dst2 = np.asarray(a_src2, np.float32), np.asarray(a_dst2, np.float32)
    b1, b2 = np.asarray(b1, np.float32), np.asarray(b2, np.float32)

    perm = np.empty(128, np.int64)        # perm[c*4+h] = h*32+c
    for c in range(HID):
        for h in range(HEADS):
            perm[c * 4 + h] = h * HID + c
    W1p = W1[:, perm]                               # [128, 128]
    ws1 = np.stack([W1[:, h * HID:(h + 1) * HID] @ a_src1[h] for h in range(HEADS)], 1)
    wd1 = np.stack([W1[:, h * HID:(h + 1) * HID] @ a_dst1[h] for h in range(HEADS)], 1)
    w1ext = np.concatenate([W1p, ws1, wd1], axis=1).astype(np.float32)  # [128,136]

    W2p = W2[perm, :]                               # [128, 32]
    ws2 = W2p @ a_src2[0][:, None]                  # [128, 1]
    wd2 = W2p @ a_dst2[0][:, None]
    w2ext = np.concatenate([W2p, ws2, wd2], axis=1).astype(np.float32)  # [128,34]

    b1p = b1[perm]
    b1r = np.broadcast_to(b1p, (128, 128)).copy().astype(np.float32)
    b2r = np.broadcast_to(b2, (128, 32)).copy().astype(np.float32)
    iotar = np.broadcast_to(np.arange(128, dtype=np.float32), (128, 128)).copy()
    identr = np.eye(128, dtype=np.float32)
    onesr = np.ones((128, 1), np.float32)
    return dict(w1ext=w1ext, w2ext=w2ext, b1r=b1r, b2r=b2r, iotar=iotar,
                identr=identr, onesr=onesr)


def make_edge_inputs(cfg: Cfg, edge_index):
    """Per-core padded window metadata. Returns list of dicts per core."""
    N, E = cfg.N, edge_index.shape[1]
    src = np.concatenate([edge_index[0], np.arange(N)]).astype(np.int64)
    dst = np.concatenate([edge_index[1], np.arange(N)]).astype(np.int64)
    order = np.argsort(dst, kind="stable")
    src, dst = src[order], dst[order]

    c = dst // cfg.npc
    loc = dst - c * cfg.npc
    wv = loc // cfg.WIN
    rel = loc - wv * cfg.WIN
    gw = c * cfg.nwin + wv
    ngw = cfg.n_cores * cfg.nwin
    counts = np.bincount(gw, minlength=ngw)
    tmax_needed = int(np.ceil(counts.max() / 128))
    assert tmax_needed <= cfg.TMAX, f"TMAX {cfg.TMAX} < needed {tmax_needed}"
    starts = np.zeros(ngw, np.int64)
    starts[1:] = np.cumsum(counts)[:-1]
    pos = np.arange(len(src)) - starts[gw]
    flat = gw * (cfg.TMAX * 128) + pos
    tot = ngw * cfg.TMAX * 128
    msrc = np.zeros(tot, np.int32)
    msrc[flat] = src
    mrel = np.full(tot, -1.0, np.float32)
    mrel[flat] = rel

    per_core = []
    for cc in range(cfg.n_cores):
        s = cc * cfg.nwin * cfg.TMAX * 128
        e = (cc + 1) * cfg.nwin * cfg.TMAX * 128
        wr = (cc * cfg.npc
              + np.arange(cfg.nwin * 128, dtype=np.int64)).clip(max=N - 1)
        per_core.append(dict(
            msrc=msrc[s:e, None].copy(),
            mrel=mrel[s:e, None].copy(),
            wrows=wr.astype(np.int32)[:, None],
        ))
    return per_core


def make_x_input(cfg: Cfg, x):
    x = np.asarray(x, np.float32)
    xp = np.zeros((cfg.Npad, IN_CH), np.float32)
    xp[:cfg.N] = x
    return np.ascontiguousarray(xp.T)


# ===================== SPMD runner =====================
import time
import numpy as np
import jax
from jax.sharding import Mesh, PartitionSpec
from jax.experimental.shard_map import shard_map

import concourse.bass as bass
from concourse import bass2jax


class SpmdRunner:
    def __init__(self, nc: bass.Bass, n_cores: int = 8):
        bass2jax.install_neuronx_cc_hook()
        if not nc.is_finalized():
            nc.finalize()
        self.nc = nc
        self.n_cores = n_cores

        in_names, out_names, out_avals, zero_outs = [], [], [], []
        partition_name = nc.partition_id_tensor.name if nc.partition_id_tensor else None
        import concourse.mybir as mybir
        for alloc in nc.m.functions[0].allocations:
            if not isinstance(alloc, mybir.MemoryLocationSet):
                continue
            name = alloc.memorylocations[0].name
            if alloc.kind == "ExternalInput":
                if name != partition_name:
                    in_names.append(name)
            elif alloc.kind == "ExternalOutput":
                out_names.append(name)
                shape = tuple(alloc.tensor_shape)
                dtype = mybir.dt.np(alloc.dtype)
                out_avals.append(jax.core.ShapedArray(shape, dtype))
                zero_outs.append(np.zeros(shape, dtype))
        self.in_names = list(in_names)
        self.out_names = out_names
        self.out_avals = out_avals
        self.zero_outs = zero_outs
        n_params = len(in_names)
        n_outs = len(out_avals)
        all_in_names = in_names + out_names
        if partition_name is not None:
            all_in_names.append(partition_name)
        self.partition_name = partition_name

        donate = tuple(range(n_params, n_params + n_outs))

        def _body(*args):
            operands = list(args)
            if partition_name is not None:
                operands.append(bass2jax.partition_id_tensor())
            outs = bass2jax._bass_exec_p.bind(
                *operands,
                out_avals=tuple(out_avals),
                in_names=tuple(all_in_names),
                out_names=tuple(out_names),
                lowering_input_output_aliases=(),
                sim_require_finite=True,
                sim_require_nnan=True,
                nc=nc,
            )
            return tuple(outs)

        devices = jax.devices()[:n_cores]
        self.mesh = Mesh(np.asarray(devices), ("core",))
        in_specs = (PartitionSpec("core"),) * (n_params + n_outs)
        out_specs = (PartitionSpec("core"),) * len(out_names)
        self.fn = jax.jit(
            shard_map(_body, mesh=self.mesh, in_specs=in_specs, out_specs=out_specs,
                      check_rep=False),
            donate_argnums=donate, keep_unused=True,
        )

    def put_inputs(self, in_maps: list[dict[str, np.ndarray]]):
        """Concat per-core inputs along axis 0 and device_put once."""
        n = self.n_cores
        concat_in = [
            np.ascontiguousarray(
                np.concatenate([np.asarray(in_maps[c][name]) for c in range(n)], axis=0))
            for name in self.in_names
        ]
        sharding = jax.sharding.NamedSharding(self.mesh, PartitionSpec("core"))
        return [jax.device_put(a, sharding) for a in concat_in]

    def _put_zeros(self):
        n = self.n_cores
        sharding = jax.sharding.NamedSharding(
            self.mesh, PartitionSpec("core"))
        return [jax.device_put(
            np.zeros((n * z.shape[0], *z.shape[1:]), z.dtype), sharding)
            for z in self.zero_outs]

    def run(self, dev_inputs, zeros=None):
        if zeros is None:
            zeros = self._put_zeros()
        out = self.fn(*dev_inputs, *zeros)
        jax.block_until_ready(out)
        return out

    def run_timed(self, dev_inputs, iters=5):
        # warmup (includes compile)
        out = self.run(dev_inputs)
        times = []
        for _ in range(iters):
            zeros = self._put_zeros()
            jax.block_until_ready(zeros)
            t0 = time.perf_counter()
            out = self.fn(*dev_inputs, *zeros)
            jax.block_until_ready(out)
            t1 = time.perf_counter()
            times.append(t1 - t0)
        return out, times

    def results(self, out_arrs):
        n = self.n_cores
        return [
            {name: np.asarray(out_arrs[i]).reshape(n, *self.out_avals[i].shape)[c]
             for i, name in enumerate(self.out_names)}
            for c in range(n)
        ]


# ===================== public entry =====================

N_FULL = 100000
_CACHE = {}


def _get():
    if "r" not in _CACHE:
        cfg = Cfg(N_FULL, n_cores=8, TMAX=19)
        nc = build_gat(cfg)
        _CACHE["r"] = (cfg, SpmdRunner(nc, cfg.n_cores))
    return _CACHE["r"]


def kernel(x, edge_index, W1, a_src1, a_dst1, b1, W2, a_src2, a_dst2, b2):
    cfg, r = _get()
    ei = np.asarray(edge_index).astype(np.int64)
    common = make_weight_inputs(W1, a_src1, a_dst1, b1, W2, a_src2, a_dst2, b2)
    common["xT"] = make_x_input(cfg, np.asarray(x, np.float32))
    per_core = make_edge_inputs(cfg, ei)
    in_maps = [dict(common, **pc) for pc in per_core]
    dev = r.put_inputs(in_maps)
    res = r.results(r.run(dev))
    out = np.concatenate([res[c]["out"] for c in range(cfg.n_cores)], axis=0)
    return out.astype(np.float32)
